# revision 1
# baseline (speedup 1.0000x reference)
import os
import sys

for _p in ("/opt/trn_rl_repo", "/root/.axon_site/_ro/trn_rl_repo"):
    if os.path.isdir(_p) and _p not in sys.path:
        sys.path.insert(0, _p)

import numpy as np

HEADS, D = 12, 64
WINDOW, SHIFT = 16, 1
SCALE = D ** -0.5
B, N, DIM = 2, 2049, 768
INNER = HEADS * D  # 768
TAUG = 258  # CLS slot + tok1/dummy slot + 256 block tokens
NCORES = 8
KT = DIM // 128  # 6

# global token ranges owned by each core (block attention); all starts == 2 mod 16
STARTS = [2, 258, 514, 770, 1026, 1282, 1538, 1794]
ENDS = [258, 514, 770, 1026, 1282, 1538, 1794, 2049]

_NC_CACHE = {}


def _build_nc():
    import concourse.bass as bass
    import concourse.bacc as bacc
    import concourse.mybir as mybir
    import concourse.tile as tile

    f32 = mybir.dt.float32
    Exp = mybir.ActivationFunctionType.Exp

    nc = bacc.Bacc(None, target_bir_lowering=False)

    x_ext = nc.declare_dram_parameter("xa", (B, TAUG, DIM), f32, isOutput=False)
    wqkv_ext = nc.declare_dram_parameter("w_qkv", (DIM, 3 * INNER), f32, isOutput=False)
    wout_ext = nc.declare_dram_parameter("w_out", (INNER, DIM), f32, isOutput=False)
    bout_ext = nc.declare_dram_parameter("b_out", (128, DIM), f32, isOutput=False)
    mask_ext = nc.declare_dram_parameter("masks", (2, 128, 128), f32, isOutput=False)
    id_ext = nc.declare_dram_parameter("ident", (128, 128), f32, isOutput=False)
    out_ext = nc.declare_dram_parameter("out_tokens", (B, TAUG, DIM), f32, isOutput=True)
    clsp_ext = nc.declare_dram_parameter("cls_part", (B, INNER + HEADS), f32, isOutput=True)
    clse_ext = nc.declare_dram_parameter("cls_extra", (B, INNER + HEADS), f32, isOutput=True)

    with tile.TileContext(nc) as tc:
        with (
            tc.tile_pool(name="wpool", bufs=1) as wp,
            tc.tile_pool(name="fpool", bufs=1) as fp,
            tc.tile_pool(name="inpool", bufs=2) as ip,
            tc.tile_pool(name="spool", bufs=3) as sp,
            tc.tile_pool(name="psA", bufs=2, space="PSUM") as psA,  # big [128,512]
            tc.tile_pool(name="psB", bufs=2, space="PSUM") as psB,  # av [64,128]
            tc.tile_pool(name="psC", bufs=2, space="PSUM") as psC,  # small
            tc.tile_pool(name="psD", bufs=1, space="PSUM") as psD,  # cls accum [1,64]/[1,1]
        ):
            # ---- constants / weights ----
            ident = wp.tile([128, 128], f32, tag="ident")
            nc.sync.dma_start(ident[:], id_ext[:])
            mask_t = []
            for s in range(2):
                m = wp.tile([128, 128], f32, tag=f"mask{s}")
                nc.sync.dma_start(m[:], mask_ext[s])
                mask_t.append(m)
            ones = wp.tile([128, 1], f32, tag="ones")
            nc.vector.memset(ones[:], 1.0)
            bias_full = wp.tile([128, DIM], f32, tag="bias_full")
            nc.sync.dma_start(bias_full[:], bout_ext[:])
            w_tiles = []
            for k in range(KT):
                t = wp.tile([128, 3 * INNER], f32, tag=f"wqkv{k}")
                nc.sync.dma_start(t[:], wqkv_ext[k * 128:(k + 1) * 128, :])
                w_tiles.append(t)
            wo_t = []
            for k in range(KT):
                t = wp.tile([128, DIM], f32, tag=f"wo{k}")
                nc.sync.dma_start(t[:], wout_ext[k * 128:(k + 1) * 128, :])
                wo_t.append(t)

            for b in range(B):
                # ---- load x and transpose to feature-major xT [768, TAUG] ----
                xT = [fp.tile([128, TAUG], f32, tag=f"xT{k}", name=f"xT{k}") for k in range(KT)]
                for tt in range(3):
                    rows = 128 if tt < 2 else TAUG - 256
                    xt = ip.tile([128, DIM], f32, tag="xt")
                    nc.sync.dma_start(xt[:rows, :], x_ext[b, tt * 128:tt * 128 + rows, :])
                    for k in range(KT):
                        pt = psA.tile([128, 128], f32, tag="big")
                        nc.tensor.transpose(pt[:, :rows], xt[:rows, k * 128:(k + 1) * 128], ident[:rows, :rows])
                        nc.vector.tensor_copy(xT[k][:, tt * 128:tt * 128 + rows], pt[:, :rows])

                # ---- per-head qT, kT (feature-major, 64-part tiles) ----
                qT, kT = [], []
                for h in range(HEADS):
                    pq = psA.tile([64, TAUG], f32, tag="big")
                    for k in range(KT):
                        nc.tensor.matmul(pq[:], w_tiles[k][:, h * 64:(h + 1) * 64], xT[k][:],
                                         start=(k == 0), stop=(k == KT - 1))
                    q = fp.tile([64, TAUG], f32, tag=f"qT{h}")
                    nc.vector.tensor_copy(q[:], pq[:])
                    qT.append(q)
                    pk = psA.tile([64, TAUG], f32, tag="big")
                    for k in range(KT):
                        nc.tensor.matmul(pk[:], w_tiles[k][:, INNER + h * 64:INNER + (h + 1) * 64], xT[k][:],
                                         start=(k == 0), stop=(k == KT - 1))
                    kk = fp.tile([64, TAUG], f32, tag=f"kT{h}")
                    nc.vector.tensor_copy(kk[:], pk[:])
                    kT.append(kk)

                # ---- v token-major: two 128-token stripes (p=2..130, 130..258) + p=0:2 ----
                vs = []
                for s in range(2):
                    vt = fp.tile([128, INNER], f32, tag=f"v{s}")
                    for half, (c0, cw) in enumerate([(0, 512), (512, 256)]):
                        pv = psA.tile([128, cw], f32, tag="big")
                        for k in range(KT):
                            nc.tensor.matmul(pv[:], xT[k][:, 2 + s * 128: 2 + (s + 1) * 128],
                                             w_tiles[k][:, 2 * INNER + c0: 2 * INNER + c0 + cw],
                                             start=(k == 0), stop=(k == KT - 1))
                        nc.vector.tensor_copy(vt[:, c0:c0 + cw], pv[:])
                    vs.append(vt)
                vc = fp.tile([2, INNER], f32, tag="vc")
                for half, (c0, cw) in enumerate([(0, 512), (512, 256)]):
                    pvc = psC.tile([2, 512], f32, tag="small")
                    for k in range(KT):
                        nc.tensor.matmul(pvc[:, :cw], xT[k][:, 0:2],
                                         w_tiles[k][:, 2 * INNER + c0: 2 * INNER + c0 + cw],
                                         start=(k == 0), stop=(k == KT - 1))
                    nc.vector.tensor_copy(vc[:, c0:c0 + cw], pvc[:, :cw])

                # ---- attention (token-major AV; per-query recip via tensor_scalar) ----
                att_s = [ip.tile([128, INNER], f32, tag=f"att{s}", name=f"att{s}") for s in range(2)]
                att1row = ip.tile([1, INNER], f32, tag="att1row")
                clsacc = sp.tile([1, INNER + HEADS], f32, tag="clsacc")
                clsext = sp.tile([1, INNER + HEADS], f32, tag="clsext")
                for h in range(HEADS):
                    # CLS-as-key row: exp(q_p . k_cls) for all p
                    pcr = psC.tile([1, TAUG], f32, tag="small")
                    nc.tensor.matmul(pcr[:], kT[h][:, 0:1], qT[h][:], start=True, stop=True)
                    ecr = sp.tile([1, TAUG], f32, tag="ecr")
                    nc.scalar.activation(ecr[:], pcr[:], Exp, scale=SCALE)

                    # token-1 special (keys p=0,1; query p=1)
                    pt1 = psC.tile([2, 1], f32, tag="small")
                    nc.tensor.matmul(pt1[:], kT[h][:, 0:2], qT[h][:, 1:2], start=True, stop=True)
                    et1 = sp.tile([2, 1], f32, tag="et1")
                    nc.scalar.activation(et1[:], pt1[:], Exp, scale=SCALE)
                    pav1 = psC.tile([1, 64], f32, tag="small")
                    nc.tensor.matmul(pav1[:], et1[:], vc[:, h * 64:(h + 1) * 64], start=True, stop=True)
                    ps1 = psC.tile([1, 1], f32, tag="small")
                    nc.tensor.matmul(ps1[:], et1[:], ones[0:2, :], start=True, stop=True)
                    r1 = sp.tile([1, 1], f32, tag="r1")
                    nc.vector.reciprocal(r1[:], ps1[:])
                    nc.vector.tensor_scalar_mul(att1row[:, h * 64:(h + 1) * 64], pav1[:], r1[:])

                    # CLS-query partials over stripes (lhsT = exp column; v moving)
                    po_cls = psD.tile([1, 64], f32, tag="pocls")
                    ps_cls = psD.tile([1, 1], f32, tag="pscls")
                    for s in range(2):
                        q0 = 2 + s * 128
                        pcc = psC.tile([128, 1], f32, tag="small")
                        nc.tensor.matmul(pcc[:], kT[h][:, q0:q0 + 128], qT[h][:, 0:1], start=True, stop=True)
                        ecc = sp.tile([128, 1], f32, tag="ecc")
                        nc.scalar.activation(ecc[:], pcc[:], Exp, scale=SCALE)
                        nc.tensor.matmul(po_cls[:], ecc[:], vs[s][:, h * 64:(h + 1) * 64],
                                         start=(s == 0), stop=(s == 1))
                        nc.tensor.matmul(ps_cls[:], ecc[:], ones[:, 0:1], start=(s == 0), stop=(s == 1))
                    nc.vector.tensor_copy(clsacc[:, h * 64:(h + 1) * 64], po_cls[:])
                    nc.vector.tensor_copy(clsacc[:, INNER + h:INNER + h + 1], ps_cls[:])

                    # extra piece (keys p=0,1 for CLS query) — host uses core 0's only
                    pce = psC.tile([2, 1], f32, tag="small")
                    nc.tensor.matmul(pce[:], kT[h][:, 0:2], qT[h][:, 0:1], start=True, stop=True)
                    ece = sp.tile([2, 1], f32, tag="ece")
                    nc.scalar.activation(ece[:], pce[:], Exp, scale=SCALE)
                    pe_o = psD.tile([1, 64], f32, tag="pocls")
                    pe_s = psD.tile([1, 1], f32, tag="pscls")
                    nc.tensor.matmul(pe_o[:], ece[:], vc[:, h * 64:(h + 1) * 64], start=True, stop=True)
                    nc.tensor.matmul(pe_s[:], ece[:], ones[0:2, 0:1], start=True, stop=True)
                    nc.vector.tensor_copy(clsext[:, h * 64:(h + 1) * 64], pe_o[:])
                    nc.vector.tensor_copy(clsext[:, INNER + h:INNER + h + 1], pe_s[:])

                    # block-diagonal stripes
                    for s in range(2):
                        q0 = 2 + s * 128
                        pst = psA.tile([128, 128], f32, tag="big")
                        nc.tensor.matmul(pst[:], kT[h][:, q0:q0 + 128], qT[h][:, q0:q0 + 128],
                                         start=True, stop=True)
                        prob = sp.tile([128, 128], f32, tag="prob")
                        nc.scalar.activation(prob[:], pst[:], Exp, scale=SCALE)
                        nc.vector.tensor_mul(prob[:], prob[:], mask_t[s][:])
                        pav = psB.tile([128, 64], f32, tag="av")
                        nc.tensor.matmul(pav[:], prob[:], vs[s][:, h * 64:(h + 1) * 64], start=True, stop=False)
                        nc.tensor.matmul(pav[:], ecr[:, q0:q0 + 128], vc[0:1, h * 64:(h + 1) * 64],
                                         start=False, stop=True)
                        psums = psC.tile([128, 1], f32, tag="small")
                        nc.tensor.matmul(psums[:], prob[:], ones[:, 0:1], start=True, stop=False)
                        nc.tensor.matmul(psums[:], ecr[:, q0:q0 + 128], ones[0:1, 0:1], start=False, stop=True)
                        rec = sp.tile([128, 1], f32, tag="rec")
                        nc.vector.reciprocal(rec[:], psums[:])
                        nc.vector.tensor_scalar_mul(att_s[s][:, h * 64:(h + 1) * 64], pav[:], rec[:])

                # ---- transpose attention output to feature-major aTfm [768, TAUG] ----
                aTfm = [fp.tile([128, TAUG], f32, tag=f"aTfm{i}", name=f"aTfm{i}") for i in range(KT)]
                for i in range(KT):
                    for s in range(2):
                        q0 = 2 + s * 128
                        pt = psA.tile([128, 128], f32, tag="big", name="ptr")
                        nc.tensor.transpose(pt[:], att_s[s][:, i * 128:(i + 1) * 128], ident[:])
                        nc.vector.tensor_copy(aTfm[i][:, q0:q0 + 128], pt[:])
                    pt1r = psC.tile([128, 1], f32, tag="small")
                    nc.tensor.transpose(pt1r[:], att1row[:, i * 128:(i + 1) * 128], ident[0:1, 0:1])
                    nc.vector.tensor_copy(aTfm[i][:, 1:2], pt1r[:])

                # ---- write CLS partials ----
                nc.sync.dma_start(clsp_ext[b:b + 1, :], clsacc[:])
                nc.sync.dma_start(clse_ext[b:b + 1, :], clsext[:])

                # ---- out projection: out[p, :] = aTfm[:, p].T @ w_out + b ----
                for tt in range(3):
                    c0 = tt * 128
                    cw = 128 if tt < 2 else TAUG - 256
                    ot = ip.tile([128, DIM], f32, tag="ot")
                    for half, (f0, fw) in enumerate([(0, 512), (512, 256)]):
                        po = psA.tile([128, fw], f32, tag="big")
                        for i in range(KT):
                            nc.tensor.matmul(po[:cw, :], aTfm[i][:, c0:c0 + cw], wo_t[i][:, f0:f0 + fw],
                                             start=(i == 0), stop=(i == KT - 1))
                        nc.vector.tensor_add(ot[:cw, f0:f0 + fw], po[:cw, :], bias_full[:cw, f0:f0 + fw])
                    nc.sync.dma_start(out_ext[b, c0:c0 + cw, :], ot[:cw, :])

    nc.compile()
    return nc


def _get_nc():
    if "nc" not in _NC_CACHE:
        _NC_CACHE["nc"] = _build_nc()
    return _NC_CACHE["nc"]


def _make_masks(core):
    start = STARTS[core]
    masks = np.zeros((2, 128, 128), dtype=np.float32)
    for s in range(2):
        g = start + s * 128 + np.arange(128)  # global tokens for p = 2+128s .. +128
        real = g < 2049
        blk = (g - 2) // 16
        same = (blk[:, None] == blk[None, :]) & real[:, None] & real[None, :]
        masks[s] = same.astype(np.float32)
    return masks


def kernel(x, w_qkv, w_out, b_out):
    x = np.asarray(x, dtype=np.float32)
    w_qkv = np.asarray(w_qkv, dtype=np.float32)
    w_out = np.asarray(w_out, dtype=np.float32)
    b_out = np.asarray(b_out, dtype=np.float32)

    ident = np.eye(128, dtype=np.float32)
    in_maps = []
    for c in range(NCORES):
        xa = np.zeros((B, TAUG, DIM), dtype=np.float32)
        xa[:, 0, :] = x[:, 0, :]
        if c == 0:
            xa[:, 1, :] = x[:, 1, :]
        L = ENDS[c] - STARTS[c]
        xa[:, 2:2 + L, :] = x[:, STARTS[c]:ENDS[c], :]
        in_maps.append({
            "xa": xa,
            "w_qkv": w_qkv,
            "w_out": w_out,
            "b_out": np.tile(b_out.reshape(1, DIM), (128, 1)),
            "masks": _make_masks(c),
            "ident": ident,
        })

    from concourse.bass_utils import run_bass_kernel_spmd

    nc = _get_nc()
    res = run_bass_kernel_spmd(nc, in_maps, core_ids=list(range(NCORES))).results

    out = np.empty((B, N, DIM), dtype=np.float32)
    for c in range(NCORES):
        L = ENDS[c] - STARTS[c]
        out[:, STARTS[c]:ENDS[c], :] = res[c]["out_tokens"][:, 2:2 + L, :]
    out[:, 1, :] = res[0]["out_tokens"][:, 1, :]

    # CLS row from partial softmax stats
    for b in range(B):
        o = res[0]["cls_extra"][b].astype(np.float64).copy()
        for c in range(NCORES):
            o = o + res[c]["cls_part"][b].astype(np.float64)
            # padding keys on core c contributed exp(0)=1 to each head's sum
            o[INNER:] -= 256 - (ENDS[c] - STARTS[c])
        ov = o[:INNER].reshape(HEADS, 64)
        s = o[INNER:]  # [HEADS]
        flat = (ov / s[:, None]).reshape(INNER)  # f = h*64 + d
        out[b, 0, :] = (flat @ w_out + b_out).astype(np.float32)
    return out



# revision 3
# speedup vs baseline: 3.0963x; 3.0963x over previous
import os
import sys

for _p in ("/opt/trn_rl_repo", "/root/.axon_site/_ro/trn_rl_repo"):
    if os.path.isdir(_p) and _p not in sys.path:
        sys.path.insert(0, _p)

import numpy as np
import ml_dtypes

BF = ml_dtypes.bfloat16

HEADS, D = 12, 64
WINDOW, SHIFT = 16, 1
SCALE = D ** -0.5
B, N, DIM = 2, 2049, 768
INNER = HEADS * D  # 768
TAUG = 258  # CLS slot + tok1/dummy slot + 256 block tokens
NCORES = 8
KT = DIM // 128  # 6

# global token ranges owned by each core (block attention); all starts == 2 mod 16
STARTS = [2, 258, 514, 770, 1026, 1282, 1538, 1794]
ENDS = [258, 514, 770, 1026, 1282, 1538, 1794, 2049]

_NC_CACHE = {}


def _build_nc():
    import concourse.bass as bass
    import concourse.bacc as bacc
    import concourse.mybir as mybir
    import concourse.tile as tile

    f32 = mybir.dt.float32
    bf16 = mybir.dt.bfloat16
    Exp = mybir.ActivationFunctionType.Exp

    nc = bacc.Bacc(None, target_bir_lowering=False)

    xaT_ext = nc.declare_dram_parameter("xaT", (B, DIM, TAUG), bf16, isOutput=False)
    wqkv_ext = nc.declare_dram_parameter("w_qkv", (DIM, 3 * INNER), bf16, isOutput=False)
    wout_ext = nc.declare_dram_parameter("w_out", (INNER, DIM), bf16, isOutput=False)
    bout_ext = nc.declare_dram_parameter("b_out", (128, DIM), f32, isOutput=False)
    mask_ext = nc.declare_dram_parameter("masks", (2, 128, 130), bf16, isOutput=False)
    mcls_ext = nc.declare_dram_parameter("mask_cls", (1, TAUG), f32, isOutput=False)
    vspec_ext = nc.declare_dram_parameter("v_spec", (B, 2, INNER), bf16, isOutput=False)
    out_ext = nc.declare_dram_parameter("out_tokens", (B, 256, DIM), f32, isOutput=True)
    clsn_ext = nc.declare_dram_parameter("cls_num", (B, D, HEADS), f32, isOutput=True)
    clsd_ext = nc.declare_dram_parameter("cls_den", (B, 1, HEADS), f32, isOutput=True)

    with tile.TileContext(nc) as tc:
        with (
            tc.tile_pool(name="wpool", bufs=1) as wp,
            tc.tile_pool(name="fpool", bufs=2) as fp,
            tc.tile_pool(name="inpool", bufs=2) as ip,
            tc.tile_pool(name="spool", bufs=3) as sp,
            tc.tile_pool(name="psBig", bufs=2, space="PSUM") as psBig,
            tc.tile_pool(name="psSc", bufs=2, space="PSUM") as psSc,
            tc.tile_pool(name="psAv", bufs=2, space="PSUM") as psAv,
            tc.tile_pool(name="psDn", bufs=2, space="PSUM") as psDn,
        ):
            # ---- constants / weights (loaded once) ----
            masks_t = []
            for s in range(2):
                m = wp.tile([128, 130], bf16, tag=f"mask{s}", name=f"mask{s}")
                nc.sync.dma_start(m[:], mask_ext[s])
                masks_t.append(m)
            mclst = wp.tile([1, TAUG], f32, tag="mcls")
            nc.sync.dma_start(mclst[:], mcls_ext[:])
            ones = wp.tile([128, 1], bf16, tag="ones")
            nc.vector.memset(ones[:], 1.0)
            bias_t = wp.tile([128, DIM], f32, tag="bias")
            nc.sync.dma_start(bias_t[:], bout_ext[:])
            W = []
            for k in range(KT):
                t = wp.tile([128, 3 * INNER], bf16, tag=f"wqkv{k}", name=f"wqkv{k}")
                nc.sync.dma_start(t[:], wqkv_ext[k * 128:(k + 1) * 128, :])
                W.append(t)
            WO = []
            for k in range(KT):
                t = wp.tile([128, DIM], bf16, tag=f"wo{k}", name=f"wo{k}")
                nc.sync.dma_start(t[:], wout_ext[k * 128:(k + 1) * 128, :])
                WO.append(t)

            for b in range(B):
                # ---- load xT (feature-major, host pre-transposed) ----
                X = []
                for k in range(KT):
                    t = ip.tile([128, TAUG], bf16, tag=f"xT{k}", name=f"xT{k}")
                    nc.sync.dma_start(t[:], xaT_ext[b, k * 128:(k + 1) * 128, :])
                    X.append(t)
                vspec_t = ip.tile([2, INNER], bf16, tag="vspec")
                nc.sync.dma_start(vspec_t[:], vspec_ext[b])

                # ---- q/k feature-major: QK[fo] [128, TAUG]; fo 0-5 = q, 6-11 = k ----
                QK = []
                for fo in range(12):
                    P = psBig.tile([128, TAUG], f32, tag="big")
                    for fi in range(KT):
                        nc.tensor.matmul(P[:], W[fi][:, fo * 128:(fo + 1) * 128], X[fi][:],
                                         start=(fi == 0), stop=(fi == KT - 1))
                    t = fp.tile([128, TAUG], bf16, tag=f"qk{fo}", name=f"qk{fo}")
                    nc.vector.tensor_copy(t[:], P[:])
                    QK.append(t)

                # ---- v token-major: two 128-token stripes ----
                Vsb = []
                for s in range(2):
                    vt = fp.tile([128, INNER], bf16, tag=f"v{s}", name=f"v{s}")
                    for (c0, cw) in ((0, 512), (512, 256)):
                        P = psBig.tile([128, cw], f32, tag="big")
                        for fi in range(KT):
                            nc.tensor.matmul(P[:], X[fi][:, 2 + 128 * s: 2 + 128 * (s + 1)],
                                             W[fi][:, 2 * INNER + c0: 2 * INNER + c0 + cw],
                                             start=(fi == 0), stop=(fi == KT - 1))
                        nc.vector.tensor_copy(vt[:, c0:c0 + cw], P[:])
                    Vsb.append(vt)

                # ---- attention per head; avT accumulated feature-major ----
                aTfm = [fp.tile([128, TAUG], bf16, tag=f"a{i}", name=f"aTfm{i}") for i in range(KT)]
                clsn = sp.tile([D, HEADS], f32, tag="clsn")
                clsd = sp.tile([1, HEADS], f32, tag="clsd")
                for h in range(HEADS):
                    r0 = (h % 2) * 64
                    qh = QK[h // 2][r0:r0 + 64, :]
                    kh = QK[6 + h // 2][r0:r0 + 64, :]

                    # CLS-as-key row over all queries
                    pe = psSc.tile([1, TAUG], f32, tag="sc", name="ecr")
                    nc.tensor.matmul(pe[:], kh[:, 0:1], qh[:], start=True, stop=True)
                    esb = sp.tile([1, TAUG], f32, tag="esb")
                    nc.scalar.activation(esb[:], pe[:], Exp, scale=SCALE)
                    nc.vector.tensor_mul(esb[:], esb[:], mclst[:])
                    ebf = sp.tile([1, TAUG], bf16, tag="ebf")
                    nc.vector.tensor_copy(ebf[:], esb[:])

                    B_ = psAv.tile([D, TAUG], f32, tag="av")
                    nc.tensor.matmul(B_[:], vspec_t[0:1, h * 64:(h + 1) * 64], ebf[:],
                                     start=True, stop=False)
                    D_ = psDn.tile([1, TAUG], f32, tag="dn")
                    for s in range(2):
                        q0, qw = (0, 130) if s == 0 else (130, 128)
                        A_ = psSc.tile([128, 130], f32, tag="sc", name="scores")
                        nc.tensor.matmul(A_[:, :qw], kh[:, 2 + 128 * s: 130 + 128 * s],
                                         qh[:, q0:q0 + qw], start=True, stop=True)
                        prob = sp.tile([128, 130], bf16, tag="prob")
                        nc.scalar.activation(prob[:, :qw], A_[:, :qw], Exp, scale=SCALE)
                        nc.vector.tensor_mul(prob[:, :qw], prob[:, :qw], masks_t[s][:, :qw])
                        nc.tensor.matmul(B_[:, q0:q0 + qw], Vsb[s][:, h * 64:(h + 1) * 64],
                                         prob[:, :qw], start=False, stop=(s == 1))
                        nc.tensor.matmul(D_[:, q0:q0 + qw], ones[:], prob[:, :qw],
                                         start=(s == 0), stop=(s == 1))
                    dsb = sp.tile([1, TAUG], f32, tag="dsb")
                    nc.vector.tensor_add(dsb[:], D_[:], esb[:])
                    rec = sp.tile([1, TAUG], f32, tag="rec")
                    nc.vector.reciprocal(rec[:], dsb[:])
                    recb = sp.tile([D, TAUG], f32, tag="recb")
                    nc.gpsimd.partition_broadcast(recb[:], rec[:])
                    nc.vector.tensor_mul(aTfm[h // 2][r0:r0 + 64, :], B_[:], recb[:])
                    nc.vector.tensor_copy(clsn[:, h:h + 1], B_[:, 0:1])
                    nc.vector.tensor_copy(clsd[:, h:h + 1], dsb[:, 0:1])

                nc.sync.dma_start(clsn_ext[b], clsn[:])
                nc.sync.dma_start(clsd_ext[b], clsd[:])

                # ---- out projection over the 256 block tokens ----
                for t in range(2):
                    c0 = 2 + 128 * t
                    osb = ip.tile([128, DIM], f32, tag="osb")
                    for (f0, fw) in ((0, 512), (512, 256)):
                        PO = psBig.tile([128, fw], f32, tag="big")
                        for i in range(KT):
                            nc.tensor.matmul(PO[:], aTfm[i][:, c0:c0 + 128], WO[i][:, f0:f0 + fw],
                                             start=(i == 0), stop=(i == KT - 1))
                        nc.vector.tensor_add(osb[:, f0:f0 + fw], PO[:], bias_t[:, f0:f0 + fw])
                    nc.sync.dma_start(out_ext[b, 128 * t:128 * t + 128, :], osb[:])

    nc.compile()
    return nc


def _get_nc():
    if "nc" not in _NC_CACHE:
        _NC_CACHE["nc"] = _build_nc()
    return _NC_CACHE["nc"]


def _make_masks(core):
    start = STARTS[core]
    m = np.zeros((2, 128, 130), dtype=np.float32)
    for s in range(2):
        g = start + s * 128 + np.arange(128)  # global token ids of this stripe's keys
        real = g < 2049
        blk = (g - 2) // 16
        bd = (blk[:, None] == blk[None, :]) & real[:, None] & real[None, :]
        if s == 0:
            m[0, :, 0] = real.astype(np.float32)  # CLS query attends all real keys
            m[0, :, 2:130] = bd
        else:
            m[1, :, 0:128] = bd
    return m


def make_in_maps(x, w_qkv, w_out, b_out):
    x = np.asarray(x, dtype=np.float32)
    w_qkv = np.asarray(w_qkv, dtype=np.float32)
    w_out = np.asarray(w_out, dtype=np.float32)
    b_out = np.asarray(b_out, dtype=np.float32)
    w_v = w_qkv[:, 2 * INNER:]
    wqkv_bf = w_qkv.astype(BF)
    wout_bf = w_out.astype(BF)
    bias128 = np.tile(b_out.reshape(1, DIM), (128, 1)).astype(np.float32)
    in_maps = []
    for c in range(NCORES):
        xa = np.zeros((B, TAUG, DIM), dtype=np.float32)
        xa[:, 0, :] = x[:, 0, :]
        if c == 0:
            xa[:, 1, :] = x[:, 1, :]
        L = ENDS[c] - STARTS[c]
        xa[:, 2:2 + L, :] = x[:, STARTS[c]:ENDS[c], :]
        xaT = np.ascontiguousarray(xa.transpose(0, 2, 1)).astype(BF)
        mask_cls = np.ones((1, TAUG), dtype=np.float32)
        if c > 0:
            mask_cls[0, 0] = 0.0
        v_spec = (xa[:, 0:2, :] @ w_v).astype(BF)
        in_maps.append({
            "xaT": xaT,
            "w_qkv": wqkv_bf,
            "w_out": wout_bf,
            "b_out": bias128,
            "masks": _make_masks(c).astype(BF),
            "mask_cls": mask_cls,
            "v_spec": v_spec,
        })
    return in_maps


def kernel(x, w_qkv, w_out, b_out):
    x = np.asarray(x, dtype=np.float32)
    w_qkv = np.asarray(w_qkv, dtype=np.float32)
    w_out = np.asarray(w_out, dtype=np.float32)
    b_out = np.asarray(b_out, dtype=np.float32)

    in_maps = make_in_maps(x, w_qkv, w_out, b_out)

    from concourse.bass_utils import run_bass_kernel_spmd

    nc = _get_nc()
    res = run_bass_kernel_spmd(nc, in_maps, core_ids=list(range(NCORES))).results

    out = np.empty((B, N, DIM), dtype=np.float32)
    for c in range(NCORES):
        L = ENDS[c] - STARTS[c]
        out[:, STARTS[c]:ENDS[c], :] = res[c]["out_tokens"][:, :L, :]

    # host-side special rows (CLS = global token 0, tok1 = global token 1)
    w_q = w_qkv[:, :INNER].astype(np.float64)
    w_k = w_qkv[:, INNER:2 * INNER].astype(np.float64)
    w_v = w_qkv[:, 2 * INNER:].astype(np.float64)
    w_o = w_out.astype(np.float64)
    b_o = b_out.astype(np.float64)
    for b in range(B):
        x0 = x[b, 0].astype(np.float64)
        x1 = x[b, 1].astype(np.float64)
        q_cls = (x0 @ w_q).reshape(HEADS, D)
        q_t1 = (x1 @ w_q).reshape(HEADS, D)
        k_cls = (x0 @ w_k).reshape(HEADS, D)
        k_t1 = (x1 @ w_k).reshape(HEADS, D)
        v_cls = (x0 @ w_v).reshape(HEADS, D)
        v_t1 = (x1 @ w_v).reshape(HEADS, D)

        # CLS row: sum device partials over cores, add tok1-as-key term on host
        num = np.zeros((D, HEADS), dtype=np.float64)
        den = np.zeros(HEADS, dtype=np.float64)
        for c in range(NCORES):
            num += res[c]["cls_num"][b].astype(np.float64)
            den += res[c]["cls_den"][b][0].astype(np.float64)
        e_t1 = np.exp(SCALE * (q_cls * k_t1).sum(1))  # [HEADS]
        num += v_t1.T * e_t1[None, :]
        den += e_t1
        att = (num / den[None, :]).T.reshape(INNER)  # f = h*64 + d
        out[b, 0, :] = (att @ w_o + b_o).astype(np.float32)

        # tok1 row: attends {CLS, itself} only — fully host-computed
        e_c = np.exp(SCALE * (q_t1 * k_cls).sum(1))
        e_1 = np.exp(SCALE * (q_t1 * k_t1).sum(1))
        att1 = ((e_c[:, None] * v_cls + e_1[:, None] * v_t1)
                / (e_c + e_1)[:, None]).reshape(INNER)
        out[b, 1, :] = (att1 @ w_o + b_o).astype(np.float32)
    return out


# revision 6
# speedup vs baseline: 3.8670x; 1.2489x over previous
import os
import sys

for _p in ("/opt/trn_rl_repo", "/root/.axon_site/_ro/trn_rl_repo"):
    if os.path.isdir(_p) and _p not in sys.path:
        sys.path.insert(0, _p)

import numpy as np
import ml_dtypes

BF = ml_dtypes.bfloat16

HEADS, D = 12, 64
WINDOW, SHIFT = 16, 1
SCALE = D ** -0.5
B, N, DIM = 2, 2049, 768
INNER = HEADS * D  # 768
TAUG = 258  # CLS slot + tok1/dummy slot + 256 block tokens
NCORES = 8
KT = DIM // 128  # 6
VW = 65  # per-head v stride: 64 value cols + 1 ones col (denominator)

# global token ranges owned by each core (block attention); all starts == 2 mod 16
STARTS = [2, 258, 514, 770, 1026, 1282, 1538, 1794]
ENDS = [258, 514, 770, 1026, 1282, 1538, 1794, 2049]

_NC_CACHE = {}


def _build_nc():
    import concourse.bass as bass
    import concourse.bacc as bacc
    import concourse.mybir as mybir
    import concourse.tile as tile

    f32 = mybir.dt.float32
    bf16 = mybir.dt.bfloat16
    Exp = mybir.ActivationFunctionType.Exp

    nc = bacc.Bacc(None, target_bir_lowering=False)

    xaT_ext = nc.declare_dram_parameter("xaT", (B, DIM, TAUG), bf16, isOutput=False)
    wqkv_ext = nc.declare_dram_parameter("w_qkv", (DIM, 3 * INNER), bf16, isOutput=False)
    wout_ext = nc.declare_dram_parameter("w_out", (INNER, DIM), bf16, isOutput=False)
    bout_ext = nc.declare_dram_parameter("b_out", (128, DIM), f32, isOutput=False)
    mask_ext = nc.declare_dram_parameter("masks", (2, 128, 130), bf16, isOutput=False)
    mcls_ext = nc.declare_dram_parameter("mask_cls", (HEADS, TAUG), f32, isOutput=False)
    vspec_ext = nc.declare_dram_parameter("v_spec", (B, HEADS, HEADS * VW), bf16, isOutput=False)
    out_ext = nc.declare_dram_parameter("out_tokens", (B, 256, DIM), f32, isOutput=True)
    clsn_ext = nc.declare_dram_parameter("cls_num", (B, D, HEADS), f32, isOutput=True)
    clsd_ext = nc.declare_dram_parameter("cls_den", (B, 1, HEADS), f32, isOutput=True)

    with tile.TileContext(nc) as tc:
        with (
            tc.tile_pool(name="wpool", bufs=1) as wp,
            tc.tile_pool(name="fpool", bufs=2) as fp,
            tc.tile_pool(name="inpool", bufs=2) as ip,
            tc.tile_pool(name="spool", bufs=3) as sp,
            tc.tile_pool(name="psBig", bufs=2, space="PSUM") as psBig,
            tc.tile_pool(name="psSc", bufs=3, space="PSUM") as psSc,
            tc.tile_pool(name="psAv", bufs=2, space="PSUM") as psAv,
            tc.tile_pool(name="psE", bufs=1, space="PSUM") as psE,
        ):
            # ---- constants / weights (loaded once) ----
            masks_t = []
            for s in range(2):
                m = wp.tile([128, 130], bf16, tag=f"mask{s}", name=f"mask{s}")
                nc.sync.dma_start(m[:], mask_ext[s])
                masks_t.append(m)
            mclst = wp.tile([HEADS, TAUG], f32, tag="mcls")
            nc.sync.dma_start(mclst[:], mcls_ext[:])
            bias_t = wp.tile([128, DIM], f32, tag="bias")
            nc.sync.dma_start(bias_t[:], bout_ext[:])
            W = []
            for k in range(KT):
                t = wp.tile([128, 3 * INNER], bf16, tag=f"wqkv{k}", name=f"wqkv{k}")
                nc.sync.dma_start(t[:], wqkv_ext[k * 128:(k + 1) * 128, :])
                W.append(t)
            WO = []
            for k in range(KT):
                t = wp.tile([128, DIM], bf16, tag=f"wo{k}", name=f"wo{k}")
                nc.sync.dma_start(t[:], wout_ext[k * 128:(k + 1) * 128, :])
                WO.append(t)

            for b in range(B):
                # ---- load xT (feature-major, host pre-transposed) ----
                X = []
                for k in range(KT):
                    t = ip.tile([128, TAUG], bf16, tag=f"xT{k}", name=f"xT{k}")
                    nc.sync.dma_start(t[:], xaT_ext[b, k * 128:(k + 1) * 128, :])
                    X.append(t)
                vspec_t = ip.tile([HEADS, HEADS * VW], bf16, tag="vspec")
                nc.sync.dma_start(vspec_t[:], vspec_ext[b])

                # ---- q/k feature-major: QK[fo] [128, TAUG]; fo 0-5 = q, 6-11 = k ----
                QK = []
                for fo in range(12):
                    P = psBig.tile([128, TAUG], f32, tag="big")
                    for fi in range(KT):
                        nc.tensor.matmul(P[:], W[fi][:, fo * 128:(fo + 1) * 128], X[fi][:],
                                         start=(fi == 0), stop=(fi == KT - 1))
                    t = fp.tile([128, TAUG], bf16, tag=f"qk{fo}", name=f"qk{fo}")
                    nc.vector.tensor_copy(t[:], P[:])
                    QK.append(t)

                # ---- v token-major, 65-stride with ones col per head ----
                Vsb = []
                for s in range(2):
                    vt = fp.tile([128, HEADS * VW], bf16, tag=f"v{s}", name=f"v{s}")
                    for half in range(2):
                        c0, cw = (0, 512) if half == 0 else (512, 256)
                        P = psBig.tile([128, cw], f32, tag="big")
                        for fi in range(KT):
                            nc.tensor.matmul(P[:], X[fi][:, 2 + 128 * s: 2 + 128 * (s + 1)],
                                             W[fi][:, 2 * INNER + c0: 2 * INNER + c0 + cw],
                                             start=(fi == 0), stop=(fi == KT - 1))
                        for hh in range(8 if half == 0 else 4):
                            h = hh if half == 0 else 8 + hh
                            nc.vector.tensor_copy(vt[:, h * VW:h * VW + 64],
                                                  P[:, hh * 64:(hh + 1) * 64])
                    for h in range(HEADS):
                        nc.vector.memset(vt[:, h * VW + 64:h * VW + 65], 1.0)
                    Vsb.append(vt)

                # ---- batched CLS-key row: scores of k_cls vs all queries, all heads ----
                kbd = [fp.tile([128, HEADS], bf16, tag=f"kbd{i}", name=f"kbd{i}")
                       for i in range(KT)]
                for i in range(KT):
                    nc.vector.memset(kbd[i][:], 0.0)
                for h in range(HEADS):
                    r0 = (h % 2) * 64
                    nc.vector.tensor_copy(kbd[h // 2][r0:r0 + 64, h:h + 1],
                                          QK[6 + h // 2][r0:r0 + 64, 0:1])
                E = psE.tile([HEADS, TAUG], f32, tag="ecr")
                for i in range(KT):
                    nc.tensor.matmul(E[:], kbd[i][:], QK[i][:],
                                     start=(i == 0), stop=(i == KT - 1))
                esb = sp.tile([HEADS, TAUG], f32, tag="esb")
                nc.scalar.activation(esb[:], E[:], Exp, scale=SCALE)
                esb_bf = sp.tile([HEADS, TAUG], bf16, tag="esb_bf")
                nc.vector.tensor_mul(esb_bf[:], esb[:], mclst[:])

                # ---- attention per head; avT + denominator accumulated in B_ ----
                aTfm = [fp.tile([128, TAUG], bf16, tag=f"a{i}", name=f"aTfm{i}")
                        for i in range(KT)]
                clsn = sp.tile([D, HEADS], f32, tag="clsn")
                clsd = sp.tile([1, HEADS], f32, tag="clsd")
                for h in range(HEADS):
                    r0 = (h % 2) * 64
                    qh = QK[h // 2][r0:r0 + 64, :]
                    kh = QK[6 + h // 2][r0:r0 + 64, :]

                    B_ = psAv.tile([VW, TAUG], f32, tag="av")
                    # CLS-as-key via block-diagonal v_spec: row h selects head h
                    nc.tensor.matmul(B_[:], vspec_t[:, h * VW:(h + 1) * VW], esb_bf[:],
                                     start=True, stop=False)
                    for s in range(2):
                        A_ = psSc.tile([128, 130], f32, tag="sc", name="scores")
                        if s == 0:
                            nc.tensor.matmul(A_[:, 0:130], kh[:, 2:130], qh[:, 0:130],
                                             start=True, stop=True)
                            ncols = 130
                        else:
                            nc.tensor.matmul(A_[:, 0:128], kh[:, 130:258], qh[:, 130:258],
                                             start=True, stop=True)
                            # CLS query vs stripe-1 keys (col 128)
                            nc.tensor.matmul(A_[:, 128:129], kh[:, 130:258], qh[:, 0:1],
                                             start=True, stop=True, skip_group_check=True)
                            ncols = 129
                        prob = sp.tile([128, 130], bf16, tag="prob")
                        nc.scalar.activation(prob[:, :ncols], A_[:, :ncols], Exp, scale=SCALE)
                        nc.vector.tensor_mul(prob[:, :ncols], prob[:, :ncols],
                                             masks_t[s][:, :ncols])
                        if s == 0:
                            nc.tensor.matmul(B_[:, 0:130], Vsb[0][:, h * VW:(h + 1) * VW],
                                             prob[:, 0:130], start=False, stop=False)
                        else:
                            nc.tensor.matmul(B_[:, 130:258], Vsb[1][:, h * VW:(h + 1) * VW],
                                             prob[:, 0:128], start=False, stop=False)
                            nc.tensor.matmul(B_[:, 0:1], Vsb[1][:, h * VW:(h + 1) * VW],
                                             prob[:, 128:129], start=False, stop=True)
                    dsb = sp.tile([1, TAUG], f32, tag="dsb")
                    nc.vector.tensor_copy(dsb[:], B_[64:65, :])
                    rec = sp.tile([1, TAUG], f32, tag="rec")
                    nc.vector.reciprocal_approx_fast(rec[:], dsb[:])
                    recb = sp.tile([D, TAUG], f32, tag="recb")
                    nc.gpsimd.partition_broadcast(recb[:], rec[:])
                    nc.vector.tensor_mul(aTfm[h // 2][r0:r0 + 64, :], B_[0:64, :], recb[:])
                    nc.vector.tensor_copy(clsn[:, h:h + 1], B_[0:64, 0:1])
                    nc.vector.tensor_copy(clsd[:, h:h + 1], B_[64:65, 0:1])

                nc.sync.dma_start(clsn_ext[b], clsn[:])
                nc.sync.dma_start(clsd_ext[b], clsd[:])

                # ---- out projection over the 256 block tokens ----
                for t in range(2):
                    c0 = 2 + 128 * t
                    osb = ip.tile([128, DIM], f32, tag="osb")
                    for half in range(2):
                        f0, fw = (0, 512) if half == 0 else (512, 256)
                        PO = psBig.tile([128, fw], f32, tag="big")
                        for i in range(KT):
                            nc.tensor.matmul(PO[:], aTfm[i][:, c0:c0 + 128], WO[i][:, f0:f0 + fw],
                                             start=(i == 0), stop=(i == KT - 1))
                        nc.vector.tensor_add(osb[:, f0:f0 + fw], PO[:], bias_t[:, f0:f0 + fw])
                    nc.sync.dma_start(out_ext[b, 128 * t:128 * t + 128, :], osb[:])

    nc.compile()
    return nc


def _get_nc():
    if "nc" not in _NC_CACHE:
        _NC_CACHE["nc"] = _build_nc()
    return _NC_CACHE["nc"]


def _make_masks(core):
    start = STARTS[core]
    m = np.zeros((2, 128, 130), dtype=np.float32)
    for s in range(2):
        g = start + s * 128 + np.arange(128)  # global token ids of this stripe's keys
        real = g < 2049
        blk = (g - 2) // 16
        bd = (blk[:, None] == blk[None, :]) & real[:, None] & real[None, :]
        if s == 0:
            m[0, :, 0] = real.astype(np.float32)  # CLS query attends all real keys
            m[0, :, 2:130] = bd
        else:
            m[1, :, 0:128] = bd
            m[1, :, 128] = real.astype(np.float32)  # CLS query col for stripe 1
    return m


def make_in_maps(x, w_qkv, w_out, b_out):
    x = np.asarray(x, dtype=np.float32)
    w_qkv = np.asarray(w_qkv, dtype=np.float32)
    w_out = np.asarray(w_out, dtype=np.float32)
    b_out = np.asarray(b_out, dtype=np.float32)
    w_v = w_qkv[:, 2 * INNER:]
    wqkv_bf = w_qkv.astype(BF)
    wout_bf = w_out.astype(BF)
    bias128 = np.tile(b_out.reshape(1, DIM), (128, 1)).astype(np.float32)
    in_maps = []
    for c in range(NCORES):
        xa = np.zeros((B, TAUG, DIM), dtype=np.float32)
        xa[:, 0, :] = x[:, 0, :]
        if c == 0:
            xa[:, 1, :] = x[:, 1, :]
        L = ENDS[c] - STARTS[c]
        xa[:, 2:2 + L, :] = x[:, STARTS[c]:ENDS[c], :]
        xaT = np.ascontiguousarray(xa.transpose(0, 2, 1)).astype(BF)
        mask_cls = np.ones((HEADS, TAUG), dtype=np.float32)
        if c > 0:
            mask_cls[:, 0] = 0.0
        v_cls = xa[:, 0, :] @ w_v  # [B, 768]
        v_spec = np.zeros((B, HEADS, HEADS * VW), dtype=np.float32)
        for h in range(HEADS):
            v_spec[:, h, h * VW:h * VW + 64] = v_cls[:, h * 64:(h + 1) * 64]
            v_spec[:, h, h * VW + 64] = 1.0
        in_maps.append({
            "xaT": xaT,
            "w_qkv": wqkv_bf,
            "w_out": wout_bf,
            "b_out": bias128,
            "masks": _make_masks(c).astype(BF),
            "mask_cls": mask_cls,
            "v_spec": v_spec.astype(BF),
        })
    return in_maps


def kernel(x, w_qkv, w_out, b_out):
    x = np.asarray(x, dtype=np.float32)
    w_qkv = np.asarray(w_qkv, dtype=np.float32)
    w_out = np.asarray(w_out, dtype=np.float32)
    b_out = np.asarray(b_out, dtype=np.float32)

    in_maps = make_in_maps(x, w_qkv, w_out, b_out)

    from concourse.bass_utils import run_bass_kernel_spmd

    nc = _get_nc()
    res = run_bass_kernel_spmd(nc, in_maps, core_ids=list(range(NCORES))).results

    out = np.empty((B, N, DIM), dtype=np.float32)
    for c in range(NCORES):
        L = ENDS[c] - STARTS[c]
        out[:, STARTS[c]:ENDS[c], :] = res[c]["out_tokens"][:, :L, :]

    # host-side special rows (CLS = global token 0, tok1 = global token 1)
    w_q = w_qkv[:, :INNER].astype(np.float64)
    w_k = w_qkv[:, INNER:2 * INNER].astype(np.float64)
    w_v = w_qkv[:, 2 * INNER:].astype(np.float64)
    w_o = w_out.astype(np.float64)
    b_o = b_out.astype(np.float64)
    for b in range(B):
        x0 = x[b, 0].astype(np.float64)
        x1 = x[b, 1].astype(np.float64)
        q_cls = (x0 @ w_q).reshape(HEADS, D)
        q_t1 = (x1 @ w_q).reshape(HEADS, D)
        k_cls = (x0 @ w_k).reshape(HEADS, D)
        k_t1 = (x1 @ w_k).reshape(HEADS, D)
        v_cls = (x0 @ w_v).reshape(HEADS, D)
        v_t1 = (x1 @ w_v).reshape(HEADS, D)

        # CLS row: sum device partials over cores, add tok1-as-key term on host
        num = np.zeros((D, HEADS), dtype=np.float64)
        den = np.zeros(HEADS, dtype=np.float64)
        for c in range(NCORES):
            num += res[c]["cls_num"][b].astype(np.float64)
            den += res[c]["cls_den"][b][0].astype(np.float64)
        e_t1 = np.exp(SCALE * (q_cls * k_t1).sum(1))  # [HEADS]
        num += v_t1.T * e_t1[None, :]
        den += e_t1
        att = (num / den[None, :]).T.reshape(INNER)  # f = h*64 + d
        out[b, 0, :] = (att @ w_o + b_o).astype(np.float32)

        # tok1 row: attends {CLS, itself} only — fully host-computed
        e_c = np.exp(SCALE * (q_t1 * k_cls).sum(1))
        e_1 = np.exp(SCALE * (q_t1 * k_t1).sum(1))
        att1 = ((e_c[:, None] * v_cls + e_1[:, None] * v_t1)
                / (e_c + e_1)[:, None]).reshape(INNER)
        out[b, 1, :] = (att1 @ w_o + b_o).astype(np.float32)
    return out


# revision 7
# speedup vs baseline: 4.2384x; 1.0960x over previous
import os
import sys

for _p in ("/opt/trn_rl_repo", "/root/.axon_site/_ro/trn_rl_repo"):
    if os.path.isdir(_p) and _p not in sys.path:
        sys.path.insert(0, _p)

import numpy as np
import ml_dtypes

BF = ml_dtypes.bfloat16

HEADS, D = 12, 64
WINDOW, SHIFT = 16, 1
SCALE = D ** -0.5
B, N, DIM = 2, 2049, 768
INNER = HEADS * D  # 768
TAUG = 258  # CLS slot + tok1/dummy slot + 256 block tokens
NCORES = 8
KT = DIM // 128  # 6
VW = 65  # per-head v stride: 64 value cols + 1 ones col (denominator)

# global token ranges owned by each core (block attention); all starts == 2 mod 16
STARTS = [2, 258, 514, 770, 1026, 1282, 1538, 1794]
ENDS = [258, 514, 770, 1026, 1282, 1538, 1794, 2049]

_NC_CACHE = {}


def _build_nc():
    import concourse.bass as bass
    import concourse.bacc as bacc
    import concourse.mybir as mybir
    import concourse.tile as tile

    f32 = mybir.dt.float32
    bf16 = mybir.dt.bfloat16
    Exp = mybir.ActivationFunctionType.Exp
    Copy = mybir.ActivationFunctionType.Copy

    nc = bacc.Bacc(None, target_bir_lowering=False)

    xaT_ext = nc.declare_dram_parameter("xaT", (B, DIM, TAUG), bf16, isOutput=False)
    wqkv_ext = nc.declare_dram_parameter("w_qkv", (DIM, 3 * INNER), bf16, isOutput=False)
    wout_ext = nc.declare_dram_parameter("w_out", (INNER, DIM), bf16, isOutput=False)
    bout_ext = nc.declare_dram_parameter("b_out", (128, DIM), f32, isOutput=False)
    mask_ext = nc.declare_dram_parameter("masks", (128, 260), bf16, isOutput=False)
    mcls_ext = nc.declare_dram_parameter("mask_cls", (HEADS, TAUG), f32, isOutput=False)
    vspec_ext = nc.declare_dram_parameter("v_spec", (B, HEADS, HEADS * VW), bf16, isOutput=False)
    out_ext = nc.declare_dram_parameter("out_tokens", (B, 256, DIM), f32, isOutput=True)
    cls_ext = nc.declare_dram_parameter("cls_all", (B, VW, HEADS), f32, isOutput=True)

    with tile.TileContext(nc) as tc:
        with (
            tc.tile_pool(name="wpool", bufs=1) as wp,
            tc.tile_pool(name="fpool", bufs=2) as fp,
            tc.tile_pool(name="inpool", bufs=2) as ip,
            tc.tile_pool(name="spool", bufs=3) as sp,
            tc.tile_pool(name="psBig", bufs=2, space="PSUM") as psBig,
            tc.tile_pool(name="psSc", bufs=3, space="PSUM") as psSc,
            tc.tile_pool(name="psAv", bufs=2, space="PSUM") as psAv,
            tc.tile_pool(name="psE", bufs=1, space="PSUM") as psE,
        ):
            # ---- constants / weights (loaded once) ----
            mask_t = wp.tile([128, 260], bf16, tag="mask")
            nc.sync.dma_start(mask_t[:], mask_ext[:])
            mclst = wp.tile([HEADS, TAUG], f32, tag="mcls")
            nc.sync.dma_start(mclst[:], mcls_ext[:])
            bias_t = wp.tile([128, DIM], f32, tag="bias")
            nc.sync.dma_start(bias_t[:], bout_ext[:])
            W = []
            for k in range(KT):
                t = wp.tile([128, 3 * INNER], bf16, tag=f"wqkv{k}", name=f"wqkv{k}")
                nc.sync.dma_start(t[:], wqkv_ext[k * 128:(k + 1) * 128, :])
                W.append(t)
            WO = []
            for k in range(KT):
                t = wp.tile([128, DIM], bf16, tag=f"wo{k}", name=f"wo{k}")
                nc.sync.dma_start(t[:], wout_ext[k * 128:(k + 1) * 128, :])
                WO.append(t)

            for b in range(B):
                # ---- load xT (feature-major, host pre-transposed) ----
                X = []
                for k in range(KT):
                    t = ip.tile([128, TAUG], bf16, tag=f"xT{k}", name=f"xT{k}")
                    nc.sync.dma_start(t[:], xaT_ext[b, k * 128:(k + 1) * 128, :])
                    X.append(t)
                vspec_t = ip.tile([HEADS, HEADS * VW], bf16, tag="vspec")
                nc.sync.dma_start(vspec_t[:], vspec_ext[b])

                # ---- q/k feature-major; k tiles (fo 6-11) first so the CLS row
                # ---- machinery and head loop can start earlier ----
                QK = [None] * 12
                for fo in [6, 7, 8, 9, 10, 11, 0, 1, 2, 3, 4, 5]:
                    P = psBig.tile([128, TAUG], f32, tag="big")
                    for fi in range(KT):
                        nc.tensor.matmul(P[:], W[fi][:, fo * 128:(fo + 1) * 128], X[fi][:],
                                         start=(fi == 0), stop=(fi == KT - 1))
                    t = fp.tile([128, TAUG], bf16, tag=f"qk{fo}", name=f"qk{fo}")
                    nc.vector.tensor_copy(t[:], P[:])
                    QK[fo] = t

                # ---- batched CLS-key row: k_cls block-diagonal vs all queries ----
                kbd = fp.tile([128, KT * HEADS], bf16, tag="kbd")
                nc.vector.memset(kbd[:], 0.0)
                for h in range(HEADS):
                    r0 = (h % 2) * 64
                    c = (h // 2) * HEADS + h
                    nc.vector.tensor_copy(kbd[r0:r0 + 64, c:c + 1],
                                          QK[6 + h // 2][r0:r0 + 64, 0:1])
                E = psE.tile([HEADS, TAUG], f32, tag="ecr")
                for i in range(KT):
                    nc.tensor.matmul(E[:], kbd[:, i * HEADS:(i + 1) * HEADS], QK[i][:],
                                     start=(i == 0), stop=(i == KT - 1))
                esb = sp.tile([HEADS, TAUG], f32, tag="esb")
                nc.scalar.activation(esb[:], E[:], Exp, scale=SCALE)
                esb_bf = sp.tile([HEADS, TAUG], bf16, tag="esb_bf")
                nc.vector.tensor_mul(esb_bf[:], esb[:], mclst[:])

                # ---- v token-major, 65-stride with ones col per head ----
                Vsb = []
                for s in range(2):
                    vt = fp.tile([128, HEADS * VW], bf16, tag=f"v{s}", name=f"v{s}")
                    for half in range(2):
                        c0, cw, nh = (0, 512, 8) if half == 0 else (512, 256, 4)
                        P = psBig.tile([128, cw], f32, tag="big")
                        for fi in range(KT):
                            nc.tensor.matmul(P[:], X[fi][:, 2 + 128 * s: 2 + 128 * (s + 1)],
                                             W[fi][:, 2 * INNER + c0: 2 * INNER + c0 + cw],
                                             start=(fi == 0), stop=(fi == KT - 1))
                        dst = vt[:, half * 8 * VW: half * 8 * VW + nh * VW]
                        dst = dst.rearrange("p (h c) -> p h c", c=VW)[:, :, 0:64]
                        src = P[:].rearrange("p (h c) -> p h c", c=64)
                        nc.vector.tensor_copy(dst, src)
                    nc.vector.memset(vt[:, 64:HEADS * VW:VW], 1.0)
                    Vsb.append(vt)

                # ---- attention per head; avT + denominator accumulated in B_ ----
                aTfm = [fp.tile([128, TAUG], bf16, tag=f"a{i}", name=f"aTfm{i}")
                        for i in range(KT)]
                cls_all = sp.tile([VW, HEADS], f32, tag="cls_all")
                for h in range(HEADS):
                    r0 = (h % 2) * 64
                    qh = QK[h // 2][r0:r0 + 64, :]
                    kh = QK[6 + h // 2][r0:r0 + 64, :]

                    B_ = psAv.tile([VW, TAUG], f32, tag="av")
                    # CLS-as-key via block-diagonal v_spec: row h selects head h
                    nc.tensor.matmul(B_[:], vspec_t[:, h * VW:(h + 1) * VW], esb_bf[:],
                                     start=True, stop=False)
                    # merged scores psum: s0 -> 0:130, s1 -> 130:258, CLS-q vs s1 keys -> 258
                    A_ = psSc.tile([128, 260], f32, tag="sc")
                    nc.tensor.matmul(A_[:, 0:130], kh[:, 2:130], qh[:, 0:130],
                                     start=True, stop=True)
                    nc.tensor.matmul(A_[:, 130:258], kh[:, 130:258], qh[:, 130:258],
                                     start=True, stop=True, skip_group_check=True)
                    nc.tensor.matmul(A_[:, 258:259], kh[:, 130:258], qh[:, 0:1],
                                     start=True, stop=True, skip_group_check=True)
                    prob = sp.tile([128, 260], bf16, tag="prob")
                    nc.scalar.activation(prob[:, 0:259], A_[:, 0:259], Exp, scale=SCALE)
                    nc.vector.tensor_mul(prob[:, 0:259], prob[:, 0:259], mask_t[:, 0:259])
                    nc.tensor.matmul(B_[:, 0:130], Vsb[0][:, h * VW:(h + 1) * VW],
                                     prob[:, 0:130], start=False, stop=False)
                    nc.tensor.matmul(B_[:, 130:258], Vsb[1][:, h * VW:(h + 1) * VW],
                                     prob[:, 130:258], start=False, stop=False)
                    nc.tensor.matmul(B_[:, 0:1], Vsb[1][:, h * VW:(h + 1) * VW],
                                     prob[:, 258:259], start=False, stop=True)

                    dsb = sp.tile([1, TAUG], f32, tag="dsb")
                    nc.scalar.activation(dsb[:], B_[64:65, :], Copy)
                    rec = sp.tile([1, TAUG], f32, tag="rec")
                    nc.vector.reciprocal_approx_fast(rec[:], dsb[:])
                    recb = sp.tile([D, TAUG], f32, tag="recb")
                    nc.gpsimd.partition_broadcast(recb[:], rec[:])
                    nc.vector.tensor_mul(aTfm[h // 2][r0:r0 + 64, :], B_[0:64, :], recb[:])
                    nc.vector.tensor_copy(cls_all[:, h:h + 1], B_[:, 0:1])

                nc.sync.dma_start(cls_ext[b], cls_all[:])

                # ---- out projection over the 256 block tokens ----
                for t in range(2):
                    c0 = 2 + 128 * t
                    osb = ip.tile([128, DIM], f32, tag="osb")
                    for half in range(2):
                        f0, fw = (0, 512) if half == 0 else (512, 256)
                        PO = psBig.tile([128, fw], f32, tag="big")
                        for i in range(KT):
                            nc.tensor.matmul(PO[:], aTfm[i][:, c0:c0 + 128], WO[i][:, f0:f0 + fw],
                                             start=(i == 0), stop=(i == KT - 1))
                        nc.vector.tensor_add(osb[:, f0:f0 + fw], PO[:], bias_t[:, f0:f0 + fw])
                    nc.sync.dma_start(out_ext[b, 128 * t:128 * t + 128, :], osb[:])

    nc.compile()
    return nc


def _get_nc():
    if "nc" not in _NC_CACHE:
        _NC_CACHE["nc"] = _build_nc()
    return _NC_CACHE["nc"]


def _make_masks(core):
    start = STARTS[core]
    m = np.zeros((128, 260), dtype=np.float32)
    for s in range(2):
        g = start + s * 128 + np.arange(128)  # global token ids of this stripe's keys
        real = g < 2049
        blk = (g - 2) // 16
        bd = (blk[:, None] == blk[None, :]) & real[:, None] & real[None, :]
        if s == 0:
            m[:, 0] = real.astype(np.float32)  # CLS query attends all real s0 keys
            m[:, 2:130] = bd
        else:
            m[:, 130:258] = bd
            m[:, 258] = real.astype(np.float32)  # CLS query col vs s1 keys
    return m


def make_in_maps(x, w_qkv, w_out, b_out):
    x = np.asarray(x, dtype=np.float32)
    w_qkv = np.asarray(w_qkv, dtype=np.float32)
    w_out = np.asarray(w_out, dtype=np.float32)
    b_out = np.asarray(b_out, dtype=np.float32)
    w_v = w_qkv[:, 2 * INNER:]
    wqkv_bf = w_qkv.astype(BF)
    wout_bf = w_out.astype(BF)
    bias128 = np.tile(b_out.reshape(1, DIM), (128, 1)).astype(np.float32)
    in_maps = []
    for c in range(NCORES):
        xa = np.zeros((B, TAUG, DIM), dtype=np.float32)
        xa[:, 0, :] = x[:, 0, :]
        if c == 0:
            xa[:, 1, :] = x[:, 1, :]
        L = ENDS[c] - STARTS[c]
        xa[:, 2:2 + L, :] = x[:, STARTS[c]:ENDS[c], :]
        xaT = np.ascontiguousarray(xa.transpose(0, 2, 1)).astype(BF)
        mask_cls = np.ones((HEADS, TAUG), dtype=np.float32)
        if c > 0:
            mask_cls[:, 0] = 0.0
        v_cls = xa[:, 0, :] @ w_v  # [B, 768]
        v_spec = np.zeros((B, HEADS, HEADS * VW), dtype=np.float32)
        for h in range(HEADS):
            v_spec[:, h, h * VW:h * VW + 64] = v_cls[:, h * 64:(h + 1) * 64]
            v_spec[:, h, h * VW + 64] = 1.0
        in_maps.append({
            "xaT": xaT,
            "w_qkv": wqkv_bf,
            "w_out": wout_bf,
            "b_out": bias128,
            "masks": _make_masks(c).astype(BF),
            "mask_cls": mask_cls,
            "v_spec": v_spec.astype(BF),
        })
    return in_maps


def kernel(x, w_qkv, w_out, b_out):
    x = np.asarray(x, dtype=np.float32)
    w_qkv = np.asarray(w_qkv, dtype=np.float32)
    w_out = np.asarray(w_out, dtype=np.float32)
    b_out = np.asarray(b_out, dtype=np.float32)

    in_maps = make_in_maps(x, w_qkv, w_out, b_out)

    from concourse.bass_utils import run_bass_kernel_spmd

    nc = _get_nc()
    res = run_bass_kernel_spmd(nc, in_maps, core_ids=list(range(NCORES))).results

    out = np.empty((B, N, DIM), dtype=np.float32)
    for c in range(NCORES):
        L = ENDS[c] - STARTS[c]
        out[:, STARTS[c]:ENDS[c], :] = res[c]["out_tokens"][:, :L, :]

    # host-side special rows (CLS = global token 0, tok1 = global token 1)
    w_q = w_qkv[:, :INNER].astype(np.float64)
    w_k = w_qkv[:, INNER:2 * INNER].astype(np.float64)
    w_v = w_qkv[:, 2 * INNER:].astype(np.float64)
    w_o = w_out.astype(np.float64)
    b_o = b_out.astype(np.float64)
    for b in range(B):
        x0 = x[b, 0].astype(np.float64)
        x1 = x[b, 1].astype(np.float64)
        q_cls = (x0 @ w_q).reshape(HEADS, D)
        q_t1 = (x1 @ w_q).reshape(HEADS, D)
        k_cls = (x0 @ w_k).reshape(HEADS, D)
        k_t1 = (x1 @ w_k).reshape(HEADS, D)
        v_cls = (x0 @ w_v).reshape(HEADS, D)
        v_t1 = (x1 @ w_v).reshape(HEADS, D)

        # CLS row: sum device partials over cores, add tok1-as-key term on host
        num = np.zeros((D, HEADS), dtype=np.float64)
        den = np.zeros(HEADS, dtype=np.float64)
        for c in range(NCORES):
            cl = res[c]["cls_all"][b].astype(np.float64)
            num += cl[0:D, :]
            den += cl[D, :]
        e_t1 = np.exp(SCALE * (q_cls * k_t1).sum(1))  # [HEADS]
        num += v_t1.T * e_t1[None, :]
        den += e_t1
        att = (num / den[None, :]).T.reshape(INNER)  # f = h*64 + d
        out[b, 0, :] = (att @ w_o + b_o).astype(np.float32)

        # tok1 row: attends {CLS, itself} only — fully host-computed
        e_c = np.exp(SCALE * (q_t1 * k_cls).sum(1))
        e_1 = np.exp(SCALE * (q_t1 * k_t1).sum(1))
        att1 = ((e_c[:, None] * v_cls + e_1[:, None] * v_t1)
                / (e_c + e_1)[:, None]).reshape(INNER)
        out[b, 1, :] = (att1 @ w_o + b_o).astype(np.float32)
    return out


# revision 8
# speedup vs baseline: 4.9965x; 1.1789x over previous
import os
import sys

for _p in ("/opt/trn_rl_repo", "/root/.axon_site/_ro/trn_rl_repo"):
    if os.path.isdir(_p) and _p not in sys.path:
        sys.path.insert(0, _p)

import numpy as np
import ml_dtypes

BF = ml_dtypes.bfloat16

HEADS, D = 12, 64
WINDOW, SHIFT = 16, 1
SCALE = D ** -0.5
B, N, DIM = 2, 2049, 768
INNER = HEADS * D  # 768
TAUG = 258  # CLS slot + tok1/dummy slot + 256 block tokens
NCORES = 8
KT = DIM // 128  # 6
VW = 65  # per-head v stride: 64 value cols + 1 ones col (denominator)

# global token ranges owned by each core (block attention); all starts == 2 mod 16
STARTS = [2, 258, 514, 770, 1026, 1282, 1538, 1794]
ENDS = [258, 514, 770, 1026, 1282, 1538, 1794, 2049]

_NC_CACHE = {}


def _build_nc():
    import concourse.bass as bass
    import concourse.bacc as bacc
    import concourse.mybir as mybir
    import concourse.tile as tile

    f32 = mybir.dt.float32
    bf16 = mybir.dt.bfloat16
    Exp = mybir.ActivationFunctionType.Exp
    Copy = mybir.ActivationFunctionType.Copy

    nc = bacc.Bacc(None, target_bir_lowering=False)

    xaT_ext = nc.declare_dram_parameter("xaT", (B, DIM, TAUG), bf16, isOutput=False)
    wqkv_ext = nc.declare_dram_parameter("w_qkv", (DIM, 3 * INNER), bf16, isOutput=False)
    wout_ext = nc.declare_dram_parameter("w_out", (INNER, DIM), bf16, isOutput=False)
    bout_ext = nc.declare_dram_parameter("b_out", (128, DIM), f32, isOutput=False)
    mask_ext = nc.declare_dram_parameter("masks", (128, 260), bf16, isOutput=False)
    mcls_ext = nc.declare_dram_parameter("mask_cls", (HEADS, TAUG), f32, isOutput=False)
    vspec_ext = nc.declare_dram_parameter("v_spec", (B, HEADS, HEADS * VW), bf16, isOutput=False)
    out_ext = nc.declare_dram_parameter("out_tokens", (B, 256, DIM), f32, isOutput=True)
    cls_ext = nc.declare_dram_parameter("cls_all", (B, VW, HEADS), f32, isOutput=True)

    with tile.TileContext(nc) as tc:
        with (
            tc.tile_pool(name="wpool", bufs=1) as wp,
            tc.tile_pool(name="fpool", bufs=2) as fp,
            tc.tile_pool(name="inpool", bufs=2) as ip,
            tc.tile_pool(name="spool", bufs=3) as sp,
            tc.tile_pool(name="psBig", bufs=2, space="PSUM") as psBig,
            tc.tile_pool(name="psSc", bufs=2, space="PSUM") as psSc,
            tc.tile_pool(name="psAv", bufs=2, space="PSUM") as psAv,
            tc.tile_pool(name="psO", bufs=2, space="PSUM") as psO,
        ):
            # ---- DMA order tuned: batch-0 x first, then w_qkv by column
            # ---- group (k cols first — they gate the head loop), wo/bias last
            Xb = [[None] * KT for _ in range(B)]
            for k in range(KT):
                t = ip.tile([128, TAUG], bf16, tag=f"xT{k}", name=f"xT0_{k}")
                nc.sync.dma_start(t[:], xaT_ext[0, k * 128:(k + 1) * 128, :])
                Xb[0][k] = t
            mask_t = wp.tile([128, 260], bf16, tag="mask")
            nc.sync.dma_start(mask_t[:], mask_ext[:])
            mclst = wp.tile([HEADS, TAUG], f32, tag="mcls")
            nc.sync.dma_start(mclst[:], mcls_ext[:])
            W = [wp.tile([128, 3 * INNER], bf16, tag=f"wqkv{k}", name=f"wqkv{k}")
                 for k in range(KT)]
            for grp in (1, 0, 2):  # k cols, q cols, v cols
                for k in range(KT):
                    nc.sync.dma_start(W[k][:, grp * INNER:(grp + 1) * INNER],
                                      wqkv_ext[k * 128:(k + 1) * 128,
                                               grp * INNER:(grp + 1) * INNER])
            for k in range(KT):
                t = ip.tile([128, TAUG], bf16, tag=f"xT{k}", name=f"xT1_{k}")
                nc.sync.dma_start(t[:], xaT_ext[1, k * 128:(k + 1) * 128, :])
                Xb[1][k] = t
            bias_t = wp.tile([128, DIM], f32, tag="bias")
            nc.sync.dma_start(bias_t[:], bout_ext[:])
            WO = []
            for k in range(KT):
                t = wp.tile([128, DIM], bf16, tag=f"wo{k}", name=f"wo{k}")
                nc.sync.dma_start(t[:], wout_ext[k * 128:(k + 1) * 128, :])
                WO.append(t)

            for b in range(B):
                X = Xb[b]
                vspec_t = ip.tile([HEADS, HEADS * VW], bf16, tag="vspec")
                nc.sync.dma_start(vspec_t[:], vspec_ext[b])

                # ---- q/k feature-major; k tiles (fo 6-11) first so the CLS row
                # ---- machinery and head loop can start earlier ----
                QK = [None] * 12
                for fo in [6, 7, 8, 9, 10, 11, 0, 1, 2, 3, 4, 5]:
                    P = psBig.tile([128, TAUG], f32, tag="big")
                    for fi in range(KT):
                        nc.tensor.matmul(P[:], W[fi][:, fo * 128:(fo + 1) * 128], X[fi][:],
                                         start=(fi == 0), stop=(fi == KT - 1))
                    t = fp.tile([128, TAUG], bf16, tag=f"qk{fo}", name=f"qk{fo}")
                    nc.vector.tensor_copy(t[:], P[:])
                    QK[fo] = t

                # ---- batched CLS-key row: k_cls block-diagonal vs all queries ----
                kbd = fp.tile([128, KT * HEADS], bf16, tag="kbd")
                nc.vector.memset(kbd[:], 0.0)
                for h in range(HEADS):
                    r0 = (h % 2) * 64
                    c = (h // 2) * HEADS + h
                    nc.vector.tensor_copy(kbd[r0:r0 + 64, c:c + 1],
                                          QK[6 + h // 2][r0:r0 + 64, 0:1])
                E = psAv.tile([HEADS, TAUG], f32, tag="av", name="E")
                for i in range(KT):
                    nc.tensor.matmul(E[:], kbd[:, i * HEADS:(i + 1) * HEADS], QK[i][:],
                                     start=(i == 0), stop=(i == KT - 1))
                esb = sp.tile([HEADS, TAUG], f32, tag="esb")
                nc.scalar.activation(esb[:], E[:], Exp, scale=SCALE)
                esb_bf = sp.tile([HEADS, TAUG], bf16, tag="esb_bf")
                nc.vector.tensor_mul(esb_bf[:], esb[:], mclst[:])

                # ---- v token-major, 65-stride with ones col per head ----
                Vsb = []
                for s in range(2):
                    vt = fp.tile([128, HEADS * VW], bf16, tag=f"v{s}", name=f"v{s}")
                    for half in range(2):
                        c0, cw, nh = (0, 512, 8) if half == 0 else (512, 256, 4)
                        P = psBig.tile([128, cw], f32, tag="big")
                        for fi in range(KT):
                            nc.tensor.matmul(P[:], X[fi][:, 2 + 128 * s: 2 + 128 * (s + 1)],
                                             W[fi][:, 2 * INNER + c0: 2 * INNER + c0 + cw],
                                             start=(fi == 0), stop=(fi == KT - 1))
                        dst = vt[:, half * 8 * VW: half * 8 * VW + nh * VW]
                        dst = dst.rearrange("p (h c) -> p h c", c=VW)[:, :, 0:64]
                        src = P[:].rearrange("p (h c) -> p h c", c=64)
                        nc.vector.tensor_copy(dst, src)
                    nc.vector.memset(vt[:, 64:HEADS * VW:VW], 1.0)
                    Vsb.append(vt)

                # ---- attention in head pairs: even/odd heads hit different PE
                # ---- row groups so their score matmuls run concurrently; the
                # ---- tok-tile-0 out-projection accumulates as aTfm tiles finish
                aTfm = [fp.tile([128, TAUG], bf16, tag=f"a{i}", name=f"aTfm{i}")
                        for i in range(KT)]
                cls_all = sp.tile([VW, HEADS], f32, tag="cls_all")
                O0 = psO.tile([128, 512], f32, tag="o", name="O0")
                O1 = psO.tile([128, 256], f32, tag="o", name="O1")
                for j in range(KT):
                    ha, hb = 2 * j, 2 * j + 1
                    qa, ka = QK[j][0:64, :], QK[6 + j][0:64, :]
                    qb, kb = QK[j][64:128, :], QK[6 + j][64:128, :]

                    Ba = psAv.tile([VW, TAUG], f32, tag="av", name="Ba")
                    Bb = psAv.tile([VW, TAUG], f32, tag="av", name="Bb")
                    nc.tensor.matmul(Ba[:], vspec_t[:, ha * VW:(ha + 1) * VW], esb_bf[:],
                                     start=True, stop=False)
                    nc.tensor.matmul(Bb[:], vspec_t[:, hb * VW:(hb + 1) * VW], esb_bf[:],
                                     start=True, stop=False)
                    Aa = psSc.tile([128, 260], f32, tag="sc", name="Aa")
                    Ab = psSc.tile([128, 260], f32, tag="sc", name="Ab")
                    nc.tensor.matmul(Aa[:, 0:130], ka[:, 2:130], qa[:, 0:130],
                                     start=True, stop=True)
                    nc.tensor.matmul(Ab[:, 0:130], kb[:, 2:130], qb[:, 0:130],
                                     start=True, stop=True)
                    nc.tensor.matmul(Aa[:, 130:258], ka[:, 130:258], qa[:, 130:258],
                                     start=True, stop=True, skip_group_check=True)
                    nc.tensor.matmul(Ab[:, 130:258], kb[:, 130:258], qb[:, 130:258],
                                     start=True, stop=True, skip_group_check=True)
                    nc.tensor.matmul(Aa[:, 258:259], ka[:, 130:258], qa[:, 0:1],
                                     start=True, stop=True, skip_group_check=True)
                    nc.tensor.matmul(Ab[:, 258:259], kb[:, 130:258], qb[:, 0:1],
                                     start=True, stop=True, skip_group_check=True)
                    proba = sp.tile([128, 260], bf16, tag="proba")
                    probb = sp.tile([128, 260], bf16, tag="probb")
                    nc.scalar.activation(proba[:, 0:259], Aa[:, 0:259], Exp, scale=SCALE)
                    nc.scalar.activation(probb[:, 0:259], Ab[:, 0:259], Exp, scale=SCALE)
                    nc.vector.tensor_mul(proba[:, 0:259], proba[:, 0:259], mask_t[:, 0:259])
                    nc.vector.tensor_mul(probb[:, 0:259], probb[:, 0:259], mask_t[:, 0:259])
                    for Bx, px, h in ((Ba, proba, ha), (Bb, probb, hb)):
                        nc.tensor.matmul(Bx[:, 0:130], Vsb[0][:, h * VW:(h + 1) * VW],
                                         px[:, 0:130], start=False, stop=False)
                        nc.tensor.matmul(Bx[:, 130:258], Vsb[1][:, h * VW:(h + 1) * VW],
                                         px[:, 130:258], start=False, stop=False)
                        nc.tensor.matmul(Bx[:, 0:1], Vsb[1][:, h * VW:(h + 1) * VW],
                                         px[:, 258:259], start=False, stop=True)
                    for Bx, h, r0 in ((Ba, ha, 0), (Bb, hb, 64)):
                        dsb = sp.tile([1, TAUG], f32, tag="dsb")
                        nc.scalar.activation(dsb[:], Bx[64:65, :], Copy)
                        rec = sp.tile([1, TAUG], f32, tag="rec")
                        nc.vector.reciprocal_approx_fast(rec[:], dsb[:])
                        recb = sp.tile([D, TAUG], f32, tag="recb")
                        nc.gpsimd.partition_broadcast(recb[:], rec[:])
                        nc.vector.tensor_mul(aTfm[j][r0:r0 + 64, :], Bx[0:64, :], recb[:])
                        nc.vector.tensor_copy(cls_all[:, h:h + 1], Bx[:, 0:1])
                    # out-proj tok-tile 0 accumulates feature tile j now
                    nc.tensor.matmul(O0[:], aTfm[j][:, 2:130], WO[j][:, 0:512],
                                     start=(j == 0), stop=(j == KT - 1))
                    nc.tensor.matmul(O1[:], aTfm[j][:, 2:130], WO[j][:, 512:768],
                                     start=(j == 0), stop=(j == KT - 1))

                nc.sync.dma_start(cls_ext[b], cls_all[:])

                # ---- finish out projection: t0 psums are done; t1 dense ----
                osb0 = ip.tile([128, DIM], f32, tag="osb0")
                nc.vector.tensor_add(osb0[:, 0:512], O0[:], bias_t[:, 0:512])
                nc.vector.tensor_add(osb0[:, 512:768], O1[:], bias_t[:, 512:768])
                nc.sync.dma_start(out_ext[b, 0:128, :], osb0[:])
                osb1 = ip.tile([128, DIM], f32, tag="osb1")
                for half in range(2):
                    f0, fw = (0, 512) if half == 0 else (512, 256)
                    PO = psO.tile([128, fw], f32, tag="o", name="PO")
                    for i in range(KT):
                        nc.tensor.matmul(PO[:], aTfm[i][:, 130:258], WO[i][:, f0:f0 + fw],
                                         start=(i == 0), stop=(i == KT - 1))
                    nc.vector.tensor_add(osb1[:, f0:f0 + fw], PO[:], bias_t[:, f0:f0 + fw])
                nc.sync.dma_start(out_ext[b, 128:256, :], osb1[:])

    nc.compile()
    return nc


def _get_nc():
    if "nc" not in _NC_CACHE:
        _NC_CACHE["nc"] = _build_nc()
    return _NC_CACHE["nc"]


def _make_masks(core):
    start = STARTS[core]
    m = np.zeros((128, 260), dtype=np.float32)
    for s in range(2):
        g = start + s * 128 + np.arange(128)  # global token ids of this stripe's keys
        real = g < 2049
        blk = (g - 2) // 16
        bd = (blk[:, None] == blk[None, :]) & real[:, None] & real[None, :]
        if s == 0:
            m[:, 0] = real.astype(np.float32)  # CLS query attends all real s0 keys
            m[:, 2:130] = bd
        else:
            m[:, 130:258] = bd
            m[:, 258] = real.astype(np.float32)  # CLS query col vs s1 keys
    return m


def make_in_maps(x, w_qkv, w_out, b_out):
    x = np.asarray(x, dtype=np.float32)
    w_qkv = np.asarray(w_qkv, dtype=np.float32)
    w_out = np.asarray(w_out, dtype=np.float32)
    b_out = np.asarray(b_out, dtype=np.float32)
    w_v = w_qkv[:, 2 * INNER:]
    wqkv_bf = w_qkv.astype(BF)
    wout_bf = w_out.astype(BF)
    bias128 = np.tile(b_out.reshape(1, DIM), (128, 1)).astype(np.float32)
    in_maps = []
    for c in range(NCORES):
        xa = np.zeros((B, TAUG, DIM), dtype=np.float32)
        xa[:, 0, :] = x[:, 0, :]
        if c == 0:
            xa[:, 1, :] = x[:, 1, :]
        L = ENDS[c] - STARTS[c]
        xa[:, 2:2 + L, :] = x[:, STARTS[c]:ENDS[c], :]
        xaT = np.ascontiguousarray(xa.transpose(0, 2, 1)).astype(BF)
        mask_cls = np.ones((HEADS, TAUG), dtype=np.float32)
        if c > 0:
            mask_cls[:, 0] = 0.0
        v_cls = xa[:, 0, :] @ w_v  # [B, 768]
        v_spec = np.zeros((B, HEADS, HEADS * VW), dtype=np.float32)
        for h in range(HEADS):
            v_spec[:, h, h * VW:h * VW + 64] = v_cls[:, h * 64:(h + 1) * 64]
            v_spec[:, h, h * VW + 64] = 1.0
        in_maps.append({
            "xaT": xaT,
            "w_qkv": wqkv_bf,
            "w_out": wout_bf,
            "b_out": bias128,
            "masks": _make_masks(c).astype(BF),
            "mask_cls": mask_cls,
            "v_spec": v_spec.astype(BF),
        })
    return in_maps


def kernel(x, w_qkv, w_out, b_out):
    x = np.asarray(x, dtype=np.float32)
    w_qkv = np.asarray(w_qkv, dtype=np.float32)
    w_out = np.asarray(w_out, dtype=np.float32)
    b_out = np.asarray(b_out, dtype=np.float32)

    in_maps = make_in_maps(x, w_qkv, w_out, b_out)

    from concourse.bass_utils import run_bass_kernel_spmd

    nc = _get_nc()
    res = run_bass_kernel_spmd(nc, in_maps, core_ids=list(range(NCORES))).results

    out = np.empty((B, N, DIM), dtype=np.float32)
    for c in range(NCORES):
        L = ENDS[c] - STARTS[c]
        out[:, STARTS[c]:ENDS[c], :] = res[c]["out_tokens"][:, :L, :]

    # host-side special rows (CLS = global token 0, tok1 = global token 1)
    w_q = w_qkv[:, :INNER].astype(np.float64)
    w_k = w_qkv[:, INNER:2 * INNER].astype(np.float64)
    w_v = w_qkv[:, 2 * INNER:].astype(np.float64)
    w_o = w_out.astype(np.float64)
    b_o = b_out.astype(np.float64)
    for b in range(B):
        x0 = x[b, 0].astype(np.float64)
        x1 = x[b, 1].astype(np.float64)
        q_cls = (x0 @ w_q).reshape(HEADS, D)
        q_t1 = (x1 @ w_q).reshape(HEADS, D)
        k_cls = (x0 @ w_k).reshape(HEADS, D)
        k_t1 = (x1 @ w_k).reshape(HEADS, D)
        v_cls = (x0 @ w_v).reshape(HEADS, D)
        v_t1 = (x1 @ w_v).reshape(HEADS, D)

        # CLS row: sum device partials over cores, add tok1-as-key term on host
        num = np.zeros((D, HEADS), dtype=np.float64)
        den = np.zeros(HEADS, dtype=np.float64)
        for c in range(NCORES):
            cl = res[c]["cls_all"][b].astype(np.float64)
            num += cl[0:D, :]
            den += cl[D, :]
        e_t1 = np.exp(SCALE * (q_cls * k_t1).sum(1))  # [HEADS]
        num += v_t1.T * e_t1[None, :]
        den += e_t1
        att = (num / den[None, :]).T.reshape(INNER)  # f = h*64 + d
        out[b, 0, :] = (att @ w_o + b_o).astype(np.float32)

        # tok1 row: attends {CLS, itself} only — fully host-computed
        e_c = np.exp(SCALE * (q_t1 * k_cls).sum(1))
        e_1 = np.exp(SCALE * (q_t1 * k_t1).sum(1))
        att1 = ((e_c[:, None] * v_cls + e_1[:, None] * v_t1)
                / (e_c + e_1)[:, None]).reshape(INNER)
        out[b, 1, :] = (att1 @ w_o + b_o).astype(np.float32)
    return out
